# revision 1
# baseline (speedup 1.0000x reference)
"""Catmull-Rom activation kernel for 8 TRN2 NeuronCores.

Reference semantics (m=8192 samples, n=2048 neurons, K=10 control points):
  p0    = floor(((x+2)*6)/4 + 1), clamped to 1 at x<=-2 and 7 at x>=2
  u     = frac(2x)
  coef  = (U @ B)[:, ::-1]   with U = [u^3, u^2, u, 1]   (sample-major flat)
  Q_k   = CP[j, p0+k-1]                                  (neuron-major flat)
  out   = sum_k coef_k * Q_k elementwise ON MISMATCHED FLATTENS: at flat
          position p (sample-major), coef comes from x[p//n, p%n] while Q
          comes from neuron jq=p//m, sample iq=p%m.

Algebraic regrouping: out = ((H0*u + H1)*u + H2)*u + H3 where
  H_t = T_t[j, s],  T_t[j, v] = sum_k B[t, 3-k] * CP[j, v+k-1],  s in 1..7.

Per-core layout (core c of 8): all work happens in the neuron-major
"H layout" (256 neurons x 8192 samples). Each core receives
  xr  = x[1024c:1024(c+1), :].reshape(256, 8192)   (u side)
  xct = x[:, 256c:256(c+1)].T                       (segment side)
  cp  = control_points[256c:256(c+1), :]
and its (256, 8192) output block is exactly out rows [1024c, 1024(c+1))
reinterpreted. No collectives, no on-device transposes.

Compute structure (3 custom DVE ops, registered at import):
  CR_SEG:  d = ((clamp(x,-2,2)+2)*6)*0.25 + 1     1 inst (exact ref rounding)
  CR_FRAC: u = w + (w<0), w = 2x - rne(2x)        1 inst (magic-const round)
  CR_ACC2: acc + (d>=v)*s0 + (d>=v+1)*s1          3 insts per t (v=2,4,6)
           (s0/s1 are per-partition [P,1] table deltas; first call seeds
           the chain with the v=1 base via in1=[P,1] broadcast)
Horner runs on the Pool engine (tensor_tensor) to overlap with DVE.
"""

import sys

import numpy as np

sys.path.insert(0, "/opt/trn_rl_repo")

from contextlib import ExitStack

import concourse.bass as bass
import concourse.bacc as bacc
import concourse.mybir as mybir
from concourse import tile
from concourse import dve_ops
from concourse.dve_spec import (
    Spec, Src0, Src1, C0, C1, C2, Zero, One, maxx, minn, lower, _has_src1,
)
from concourse.dve_uop import DveOpSpec
from concourse.bass_utils import run_bass_kernel_spmd

M = 8192          # samples
N = 2048          # neurons
K = 10            # control points per neuron
NCORES = 8
NL = N // NCORES  # 256 neurons per core
P = 128           # partitions per tile
FT = 1024         # free-dim tile size
f32 = mybir.dt.float32
Alu = mybir.AluOpType
MAGIC = 12582912.0  # 1.5 * 2^23: rne-to-integer bias, valid for |t| < 2^22

# Wrev[t, k] = B[t, 3-k]; T_t[:, v] = sum_k Wrev[t,k] * CP[:, v-1+k]
_B = 0.5 * np.array(
    [[-1.0, 3.0, -3.0, 1.0],
     [2.0, -5.0, 4.0, -1.0],
     [-1.0, 0.0, 1.0, 0.0],
     [0.0, 2.0, 0.0, 0.0]], dtype=np.float32)
WREV = np.ascontiguousarray(_B[:, ::-1])  # (4, 4)

_CACHE = {}


def _register_op(name, spec):
    for o in dve_ops.OPS:
        if o.name == name:
            return o
    row = max(dve_ops._SUB_OPCODE_FOR_NAME.values()) + 1
    assert row < 0x20
    dve_ops._SUB_OPCODE_FOR_NAME[name] = row
    shas = {}
    for ver in ("v3", "v4"):
        u = lower(spec, ver=ver)
        shas[ver] = DveOpSpec(
            name=name, opcode=row, uops=u, rd1_en=_has_src1(spec)).sha(ver)
    op = dve_ops.DveOp(name, spec, subdim=False, uops_sha=shas)
    dve_ops.OPS.append(op)
    dve_ops.CUSTOM_DVE_SPECS[name] = spec
    return op


def _seg_ref(in0, in1, s0, s1, imm2):
    x = np.asarray(in0, np.float32)
    xcl = np.minimum(np.maximum(x, np.float32(-s1)), np.float32(s1))
    return (((xcl + np.float32(s1)) * np.float32(imm2)) * np.float32(s0)
            + np.float32(1.0)).astype(np.float32)


def _frac_ref(in0, in1, s0, s1, imm2):
    a = (np.asarray(in0, np.float32) * np.float32(s1)).astype(np.float32)
    r = ((a + np.float32(s0)).astype(np.float32) - np.float32(s0)).astype(np.float32)
    w = (a - r).astype(np.float32)
    return (w + (w < 0).astype(np.float32)).astype(np.float32)


def _acc2_ref(in0, in1, s0, s1, imm2):
    d = np.asarray(in0, np.float32)
    m0 = (d >= np.float32(imm2)).astype(np.float32)
    m1 = (d >= np.float32(imm2) + np.float32(1.0)).astype(np.float32)
    return (np.asarray(in1, np.float32) + m0 * np.asarray(s0, np.float32)
            + m1 * np.asarray(s1, np.float32)).astype(np.float32)


CR_SEG = _register_op("CR_SEG_ANT", Spec(
    body=((maxx(minn(Src0, C1), Zero - C1) + C1) * C2) * C0 + One,
    reference=_seg_ref))

_a = Src0 * C1
_w = _a - ((_a + C0) - C0)
CR_FRAC = _register_op("CR_FRAC_ANT", Spec(
    body=_w + (_w < Zero),
    reference=_frac_ref))

CR_ACC2 = _register_op("CR_ACC2_ANT", Spec(
    body=Src1 + (Src0 >= C2) * C0 + (Src0 >= (C2 + One)) * C1,
    reference=_acc2_ref))


def _init2_ref(in0, in1, s0, s1, imm2):
    # in1 is the C3-spilled [P,1] scalar (second delta); s1 = base
    d = np.asarray(in0, np.float32)
    m0 = (d >= np.float32(imm2)).astype(np.float32)
    m1 = (d >= np.float32(imm2) + np.float32(1.0)).astype(np.float32)
    return (np.asarray(s1, np.float32) + m0 * np.asarray(s0, np.float32)
            + m1 * np.asarray(in1, np.float32)).astype(np.float32)


from concourse.dve_spec import C3, _spill_c3_to_src1  # noqa: E402

CR_INIT2 = _register_op("CR_INIT2_ANT", Spec(
    body=_spill_c3_to_src1(
        C1 + (Src0 >= C2) * C0 + (Src0 >= (C2 + One)) * C3),
    reference=_init2_ref))


def _build_bass(gens: int = 1):
    nc = bacc.Bacc("TRN2", target_bir_lowering=False, debug=False,
                   num_devices=NCORES)
    xr = nc.dram_tensor("xr", [NL, M], f32, kind="ExternalInput").ap()
    xct = nc.dram_tensor("xct", [NL, M], f32, kind="ExternalInput").ap()
    cp = nc.dram_tensor("cp", [NL, K], f32, kind="ExternalInput").ap()
    out = nc.dram_tensor("out", [NL, M], f32, kind="ExternalOutput").ap()

    with tile.TileContext(nc, num_cores=NCORES) as tc, ExitStack() as ctx:
        const_pool = ctx.enter_context(tc.tile_pool(name="const", bufs=1))
        in_pool = ctx.enter_context(tc.tile_pool(name="inp", bufs=4))
        du_pool = ctx.enter_context(tc.tile_pool(name="du", bufs=3))
        acc_pool = ctx.enter_context(tc.tile_pool(name="acc", bufs=3))
        out_pool = ctx.enter_context(tc.tile_pool(name="outp", bufs=3))

        for gen in range(gens):
         for jb in range(NL // P):  # two 128-neuron blocks
            # ---- tiny per-block table prep (on DVE; negligible) ----
            cpt = const_pool.tile([P, K], f32, tag=f"cp{jb}")
            nc.sync.dma_start(cpt[:], cp[jb * P:(jb + 1) * P, :])
            # T_t[:, vi] for vi=0..6 (v=vi+1); D_t[:, vi] = T(vi+1)-T(vi)
            T = [const_pool.tile([P, 7], f32, tag=f"T{jb}_{t}", name=f"T{jb}_{t}")
                 for t in range(4)]
            D = [const_pool.tile([P, 6], f32, tag=f"D{jb}_{t}", name=f"D{jb}_{t}")
                 for t in range(4)]
            for t in range(4):
                nc.vector.tensor_single_scalar(
                    T[t][:], cpt[:, 0:7], float(WREV[t, 0]), Alu.mult)
                for k in range(1, 4):
                    nc.vector.scalar_tensor_tensor(
                        T[t][:], cpt[:, k:k + 7], float(WREV[t, k]), T[t][:],
                        Alu.mult, Alu.add)
                nc.vector.tensor_sub(D[t][:], T[t][:, 1:7], T[t][:, 0:6])

            for fc in range(M // FT):
                fsl = slice(fc * FT, (fc + 1) * FT)
                psl = slice(jb * P, (jb + 1) * P)

                xr_t = in_pool.tile([P, FT], f32, tag="xr")
                nc.sync.dma_start(xr_t[:], xr[psl, fsl])
                xc_t = in_pool.tile([P, FT], f32, tag="xc")
                nc.sync.dma_start(xc_t[:], xct[psl, fsl])

                # u first (Pool's first Horner op needs it), then d
                u_t = du_pool.tile([P, FT], f32, tag="u")
                nc.vector._custom_dve(CR_FRAC, out=u_t[:], in0=xr_t[:],
                                      s0=MAGIC, s1=2.0)
                d_t = du_pool.tile([P, FT], f32, tag="d")
                nc.vector._custom_dve(CR_SEG, out=d_t[:], in0=xc_t[:],
                                      s0=0.25, s1=2.0, imm2=6.0)

                # H_t = T_t[:,1] + sum_{v=2..7} (d>=v) * D_t[:,v-2]; 3 insts/t
                # (INIT2: base via s1, delta v=2 via s0, v=3 via C3-spill).
                # Horner ops (Pool) interleaved so Pool starts after each H_t.
                def hchain(t):
                    ht = acc_pool.tile([P, FT], f32, tag=f"h{t}",
                                       name=f"h{t}")
                    nc.vector._custom_dve(
                        CR_INIT2, out=ht[:], in0=d_t[:], in1=D[t][:, 1:2],
                        s0=D[t][:, 0:1], s1=T[t][:, 0:1], imm2=2.0)
                    nc.vector._custom_dve(
                        CR_ACC2, out=ht[:], in0=d_t[:], in1=ht[:],
                        s0=D[t][:, 2:3], s1=D[t][:, 3:4], imm2=4.0)
                    nc.vector._custom_dve(
                        CR_ACC2, out=ht[:], in0=d_t[:], in1=ht[:],
                        s0=D[t][:, 4:5], s1=D[t][:, 5:6], imm2=6.0)
                    return ht

                o_t = out_pool.tile([P, FT], f32, tag="o")
                h = hchain(0)
                nc.gpsimd.tensor_tensor(o_t[:], h[:], u_t[:], Alu.mult)
                h = hchain(1)
                nc.gpsimd.tensor_tensor(o_t[:], o_t[:], h[:], Alu.add)
                nc.gpsimd.tensor_tensor(o_t[:], o_t[:], u_t[:], Alu.mult)
                h = hchain(2)
                nc.gpsimd.tensor_tensor(o_t[:], o_t[:], h[:], Alu.add)
                nc.gpsimd.tensor_tensor(o_t[:], o_t[:], u_t[:], Alu.mult)
                h = hchain(3)
                nc.gpsimd.tensor_tensor(o_t[:], o_t[:], h[:], Alu.add)

                nc.sync.dma_start(out[psl, fsl], o_t[:])

    nc.finalize()
    return nc


def _get_nc():
    if "nc" not in _CACHE:
        _CACHE["nc"] = _build_bass()
    return _CACHE["nc"]


def kernel(x: np.ndarray, control_points: np.ndarray) -> np.ndarray:
    x = np.ascontiguousarray(np.asarray(x, dtype=np.float32))
    cp = np.ascontiguousarray(np.asarray(control_points, dtype=np.float32))
    assert x.shape == (M, N) and cp.shape == (N, K)

    nc = _get_nc()
    mrows = M // NCORES  # 1024 output rows per core
    in_maps = []
    for c in range(NCORES):
        xr = np.ascontiguousarray(
            x[c * mrows:(c + 1) * mrows, :]).reshape(NL, M)
        xct = np.ascontiguousarray(x[:, c * NL:(c + 1) * NL].T)
        cpc = np.ascontiguousarray(cp[c * NL:(c + 1) * NL, :])
        in_maps.append({"xr": xr, "xct": xct, "cp": cpc})

    res = run_bass_kernel_spmd(nc, in_maps, core_ids=list(range(NCORES)))
    outs = [res.results[c]["out"].reshape(mrows, N) for c in range(NCORES)]
    return np.concatenate(outs, axis=0)

